# revision 25
# baseline (speedup 1.0000x reference)
"""Trainium2 Bass kernel for nn_CandidateFinder (LSH hash-equality KNN).

Reference semantics: q/k binarized (x>0), projected by W [64,8], sign bits
packed into an 8-bit bucket code; for each query, return the first 64 key
indices (ascending) whose code equals the query's code, padded with -1.

Key insight: codes live in [0,256). Build, per batch, a [256, 64] table of
the first 64 key indices per bucket, then gather per query. Both steps map
onto matmuls + a free-dim prefix scan + one GPSIMD local_scatter.

Sharding: 8 cores = 4 batches x 2 bucket-halves (c in [0,128) / [128,256)).
Each core computes a partial gather (zero where the query's code is in the
other half); host sums the pair and subtracts 1 (table stores j+1, empty=0).

v5 structure:
- inputs host-binarized to fp16 {0,1}; no device binarize, and the hash
  matmuls start once the (halved, 2-queue) kT DMAs land.
- k-side codes: 0/1 bits via DVE is_ge from psum (the zero-padded psum rows
  become free all-ones rows); bucket match is Relu(agree + 1 - popcount(c))
  with a per-partition bias (no ACT Sign on the k critical path).
- the scatter-index mask is fused to ONE DVE op per half:
  idx = (rank - 1) + T, where T = 2048*(sum_h bit_h*pm - popcount) <= 0 is
  built by one extra matmul per chunk (pm rows scaled by 2048 plus a
  -2048*popcount coefficient on the free ones-row). T = 0 exactly at
  matches and <= -2048 otherwise, so non-matched keys go negative (ignored
  by local_scatter; ranks <= 2048 keep everything inside int16).
- fp16 iota -> fp16 tables; the per-query gather accumulates
  q1h^T @ tab0 + q1h^T @ tab1 in psum (one accumulation group per psum
  bank), with tab0 matmuls issued before scatter h1 completes.
- psum copied out in two halves (ACT + DVE) feeding SP + ACT DMA queues.

Precision: the hash sign test needs ~f32-accurate projections. W is split
as fp16(W) + fp16(W - fp16(W)) and the two fp16 matmuls accumulate in f32
PSUM; representation error ~1e-6 vs hash sign margins ~1e-4 on this data.
"""

import numpy as np

B, L, D, NH = 4, 2048, 64, 8
KMAX = 64
TABLE_ELEMS = 1024  # local_scatter num_elems; must exceed max bucket count
MPAD = 48           # hash matmul lhsT free size: 8 real + 40 zero rows
HALF = L // 2
BIG = 2048.0        # T-mask scale; > max rank, and 8*BIG stays in int16
# const-pack columns: wpk [0:96], sgnc [96:224], biasc [224], tpk [225:353]
CW, CS, CB, CT = 0, 2 * MPAD, 2 * MPAD + 128, 2 * MPAD + 129
CPACK = CT + 128

_cache = {}


def _build_program():
    import concourse.bass as bass
    import concourse.mybir as mybir
    from concourse import bacc, tile
    from contextlib import ExitStack

    dt = mybir.dt
    Alu = mybir.AluOpType
    Act = mybir.ActivationFunctionType

    nc = bacc.Bacc("TRN2", target_bir_lowering=False, debug=False)

    # DRAM I/O (per-core shapes); qT/kT are host-binarized {0,1} fp16
    qT_d = nc.declare_dram_parameter("qT", [D, L], dt.float16, isOutput=False)
    kT_d = nc.declare_dram_parameter("kT", [D, L], dt.float16, isOutput=False)
    cpk_d = nc.declare_dram_parameter("cpk", [128, CPACK], dt.float16, isOutput=False)
    out_d = nc.declare_dram_parameter("out", [L, KMAX], dt.float16, isOutput=True)

    with ExitStack() as ctx:
        tc = ctx.enter_context(tile.TileContext(nc))
        sb = ctx.enter_context(tc.tile_pool(name="sb", bufs=1))
        ps = ctx.enter_context(tc.tile_pool(name="ps", bufs=2, space="PSUM"))
        aps = ctx.enter_context(tc.tile_pool(name="aps", bufs=3, space="PSUM"))

        # ---- loads ----
        kT_sb = sb.tile([D, L], dt.float16, tag="kT")
        qT_sb = sb.tile([D, L], dt.float16, tag="qT")
        cpk_sb = sb.tile([128, CPACK], dt.float16, tag="cpk")
        biasc_sb = sb.tile([128, 1], dt.float32, tag="biasc")

        nc.sync.dma_start(kT_sb[:, 0:HALF], kT_d[:, 0:HALF])
        nc.sync.dma_start(qT_sb[:, 0:HALF], qT_d[:, 0:HALF])

        nc.gpsimd.dma_start(cpk_sb[:], cpk_d[:])
        nc.gpsimd.dma_start(kT_sb[:, HALF:L], kT_d[:, HALF:L])
        nc.gpsimd.dma_start(qT_sb[:, HALF:L], qT_d[:, HALF:L])

        wpk_sb = cpk_sb[0:D, CW : CW + 2 * MPAD]
        sgnc_sb = cpk_sb[0:MPAD, CS : CS + 128]
        tpk_sb = cpk_sb[0:MPAD, CT : CT + 128]

        # biasc as f32 (activation bias operand); tiny convert off cpk
        nc.vector.tensor_copy(biasc_sb[:], cpk_sb[:, CB : CB + 1])

        # bias constant for the q-side one-hot Relu(agree - 7)
        bias7 = sb.tile([128, 1], dt.float32, tag="bias7")
        nc.gpsimd.memset(bias7[:], -7.0)

        # iota data for the scatter: each partition holds 1..L; fp16 so the
        # scattered tables feed the gather matmuls directly (ints <= 2048
        # are exact in fp16).
        iota_sb = sb.tile([128, L], dt.float16, tag="iota")
        nc.gpsimd.iota(
            iota_sb[:], pattern=[[1, L]], base=1, channel_multiplier=0,
            allow_small_or_imprecise_dtypes=True,
        )



        # ---- PE warm-up: keep the tensor engine busy from t~1us so the
        # p-state ramp completes before the real matmuls arrive ----
        warm_src = sb.tile([D, 512], dt.float16, tag="warm")
        nc.vector.memset(warm_src[:], 0.0)
        wp = ps.tile([MPAD, 512], dt.float32, tag="hp")
        for r in range(4):
            nc.tensor.matmul(
                wp[:], lhsT=warm_src[:, 0:MPAD], rhs=warm_src[:],
                start=True, stop=True,
            )
        warm_sink = sb.tile([1, 1], dt.float32, tag="warmsink")
        nc.vector.tensor_copy(warm_sink[:], wp[0:1, 0:1])

        # ---- hash matmuls (shared helper): two fp16 matmuls (W-hi + W-lo)
        # accumulate into f32 psum; chunk pairs share a [64, 512] psum tile
        # at partition bases {0, 32}; base-0 uses zero-padded M=48 weights so
        # rows 8-31 and 40-47 are defined for the batched bits/sign op.
        def hash_pair(src_sb, g):
            t = ps.tile([64, 512], dt.float32, tag="hp")
            for u in range(2):
                c = 2 * g + u
                m = MPAD if u == 0 else NH
                nc.tensor.matmul(
                    t[32 * u : 32 * u + m, :],
                    lhsT=wpk_sb[:, 0:m], rhs=src_sb[:, 512 * c : 512 * (c + 1)],
                    start=True, stop=False,
                )
                nc.tensor.matmul(
                    t[32 * u : 32 * u + m, :],
                    lhsT=wpk_sb[:, MPAD : MPAD + m],
                    rhs=src_sb[:, 512 * c : 512 * (c + 1)],
                    start=False, stop=True,
                )
            return t

        onehot = sb.tile([128, L], dt.float16, tag="onehot")
        q1h = sb.tile([128, L], dt.float16, tag="q1h")

        # k side: hash -> DVE 0/1 bits (is_ge: psum padding rows become
        # all-ones) -> agree -> Relu(agree + 1 - popcount)
        kbits = []
        kapt = []
        for g in range(2):
            t = hash_pair(kT_sb, g)
            bits = sb.tile([MPAD, 512], dt.float16, tag=f"bitsk{g}")
            nc.vector.tensor_single_scalar(bits[:], t[0:MPAD, :], 0.0, Alu.is_ge)
            kbits.append(bits)
        for g in range(2):
            apt = aps.tile([128, 1024], dt.float32, tag="agree")
            kapt.append(apt)
            for u in range(2):
                nc.tensor.matmul(
                    apt[:, 512 * u : 512 * (u + 1)],
                    lhsT=sgnc_sb[32 * u : 32 * u + 8, :],
                    rhs=kbits[g][32 * u : 32 * u + 8, :],
                    start=True, stop=True,
                )
            nc.scalar.activation(
                onehot[:, 1024 * g : 1024 * (g + 1)], apt[:],
                Act.Relu, bias=biasc_sb[:],
            )

        # ---- rank keys within bucket (inclusive prefix sum along j) on DVE
        # (the scan opcode only exists on DVE), halves chained via the last
        # column of half 0; m1 = onehot*rank then idx = m1 - 1 (-1 at
        # non-matches = ignored by local_scatter). Half 0's mask + scatter
        # run between the two scans so scatter h0 clears the Pool queue
        # while scan h1 is still going.
        rank = sb.tile([128, L], dt.float16, tag="rank")
        m1 = sb.tile([128, L], dt.float16, tag="m1")
        idx16 = sb.tile([128, L], dt.int16, tag="idx16")
        tabs = []
        for h in range(2):
            lo, hi = HALF * h, HALF * (h + 1)
            init = 0.0 if h == 0 else rank[:, HALF - 1 : HALF]
            nc.vector.tensor_tensor_scan(
                rank[:, lo:hi], onehot[:, lo:hi], onehot[:, lo:hi],
                init, Alu.add, Alu.bypass,
            )
            nc.vector.tensor_mul(m1[:, lo:hi], onehot[:, lo:hi], rank[:, lo:hi])
            nc.vector.tensor_single_scalar(
                idx16[:, lo:hi], m1[:, lo:hi], 1.0, Alu.subtract
            )
            tab = sb.tile([128, TABLE_ELEMS], dt.float16, tag=f"table{h}")
            tabs.append(tab)
            nc.gpsimd.local_scatter(
                tab[:], iota_sb[:, lo:hi], idx16[:, lo:hi],
                channels=128, num_elems=TABLE_ELEMS, num_idxs=HALF,
            )

        # q side: hash -> ACT Sign (+-1) -> agree -> Relu(agree - 7); the q
        # pair-1 agree matmuls are held until the h0 mask frees its psum
        # slot (aps bufs=3).
        from concourse.tile_rust import add_dep_helper  # noqa: E402

        qsgn = []
        for g in range(2):
            t = hash_pair(qT_sb, g)
            s = sb.tile([MPAD, 512], dt.float16, tag=f"sgnq{g}")
            nc.scalar.activation(s[:], t[0:MPAD, :], Act.Sign)
            qsgn.append(s)
        for g in range(2):
            apt = aps.tile([128, 1024], dt.float32, tag="agree")
            for u in range(2):
                nc.tensor.matmul(
                    apt[:, 512 * u : 512 * (u + 1)],
                    lhsT=sgnc_sb[32 * u : 32 * u + 8, :],
                    rhs=qsgn[g][32 * u : 32 * u + 8, :],
                    start=True, stop=True,
                )
            nc.scalar.activation(
                q1h[:, 1024 * g : 1024 * (g + 1)], apt[:], Act.Relu, bias=bias7[:]
            )

        # ---- gather per query: out[i, s] = sum_c q1h[c, i] * tab[c, s],
        # accumulating tab0 + tab1 in psum. Queries are chunked mod-8 within
        # each 1024-half so each half only needs that half's q one-hot and
        # the output DMA rows stay per-partition contiguous. Each half has
        # its own 1-bank psum tile = one accumulation group.
        HO = 8 * KMAX
        opA = ps.tile([128, HO], dt.float32, tag="hp")
        opB = ps.tile([128, HO], dt.float32, tag="hp")
        opt = [opA, opB]
        qv = [
            q1h[:, HALF * h : HALF * (h + 1)].rearrange("c (i t) -> c t i", t=8)
            for h in range(2)
        ]
        for h in range(2):
            for ti, tab in ((0, tabs[0]), (1, tabs[1])):
                for u in range(8):
                    nc.tensor.matmul(
                        opt[h][:, KMAX * u : KMAX * (u + 1)],
                        lhsT=qv[h][:, u, :], rhs=tab[:, 0:KMAX],
                        start=(ti == 0 and u == 0), stop=(ti == 1 and u == 7),
                    )
        out_sbA = sb.tile([128, HO], dt.float16, tag="out_sbA")
        out_sbB = sb.tile([128, HO], dt.float16, tag="out_sbB")
        out_v = [
            out_d[HALF * h : HALF * (h + 1), :].rearrange("(p t) s -> p (t s)", t=8)
            for h in range(2)
        ]
        # half A (closes first): DVE copy + SP queue; half B: ACT copy + ACT
        # queue (both engines are free by the time the banks close)
        nc.vector.tensor_copy(out_sbA[:], opt[0][:])
        nc.sync.dma_start(out_v[0][:], out_sbA[:])
        nc.scalar.activation(out_sbB[:], opt[1][:], Act.Copy)
        nc.scalar.dma_start(out_v[1][:], out_sbB[:])

    nc.compile()
    return nc


def _get_nc():
    if "nc" not in _cache:
        _cache["nc"] = _build_program()
    return _cache["nc"]


def _make_in_maps(query, key, W):
    query = np.asarray(query, dtype=np.float32)
    key = np.asarray(key, dtype=np.float32)
    W = np.asarray(W, dtype=np.float32)
    qT = [
        np.ascontiguousarray((query[b].T > 0)).astype(np.float16) for b in range(B)
    ]
    kT = [
        np.ascontiguousarray((key[b].T > 0)).astype(np.float16) for b in range(B)
    ]

    wpk = np.zeros((D, 2 * MPAD), np.float16)
    wpk[:, :NH] = W.astype(np.float16)
    wpk[:, MPAD : MPAD + NH] = (W - wpk[:, :NH].astype(np.float32)).astype(np.float16)

    cpks = []
    for h in range(2):
        cg = 128 * h + np.arange(128)  # global bucket ids of this half
        bits = ((cg[None, :] >> np.arange(NH)[:, None]) & 1).astype(np.float32)
        pm = (2.0 * bits - 1.0).astype(np.float16)  # [8, 128]
        cnt = bits.sum(axis=0)  # popcount per bucket
        cpk = np.zeros((128, CPACK), np.float16)
        cpk[0:D, CW : CW + 2 * MPAD] = wpk
        cpk[0:NH, CS : CS + 128] = pm
        cpk[32 : 32 + NH, CS : CS + 128] = pm
        # match iff sum_h bit_h * pm_{h,c} == popcount(c):
        # relu(agree + 1 - popcount) is 1 at match, 0 otherwise
        cpk[:, CB] = (1.0 - cnt).astype(np.float16)
        # T-mask rows: BIG*pm on the bit rows, -BIG*popcount on the ones-row
        cpk[0:NH, CT : CT + 128] = (BIG * pm).astype(np.float16)
        cpk[NH, CT : CT + 128] = (-BIG * cnt).astype(np.float16)
        cpk[32 : 32 + NH, CT : CT + 128] = (BIG * pm).astype(np.float16)
        cpk[40, CT : CT + 128] = (-BIG * cnt).astype(np.float16)
        cpks.append(cpk)
    return [
        {"qT": qT[c // 2], "kT": kT[c // 2], "cpk": cpks[c % 2]}
        for c in range(2 * B)
    ]


def _combine(results):
    out = np.empty((B, L, KMAX), dtype=np.int64)
    for b in range(B):
        g = results[2 * b]["out"].astype(np.int64) + results[2 * b + 1]["out"].astype(
            np.int64
        )
        out[b] = g - 1
    return out


def _run_spmd(in_maps, **kwargs):
    from concourse.bass_utils import run_bass_kernel_spmd

    return run_bass_kernel_spmd(_get_nc(), in_maps, list(range(2 * B)), **kwargs)


def kernel(query, key, W, head_idx=0, **_unused):
    in_maps = _make_in_maps(query, key, W)
    res = _run_spmd(in_maps)
    return _combine(res.results)


# revision 26
# speedup vs baseline: 1.0358x; 1.0358x over previous
"""Trainium2 Bass kernel for nn_CandidateFinder (LSH hash-equality KNN).

Reference semantics: q/k binarized (x>0), projected by W [64,8], sign bits
packed into an 8-bit bucket code; for each query, return the first 64 key
indices (ascending) whose code equals the query's code, padded with -1.

Key insight: codes live in [0,256). Build, per batch, a [256, 64] table of
the first 64 key indices per bucket, then gather per query. Both steps map
onto matmuls + a free-dim prefix scan + one GPSIMD local_scatter.

Sharding: 8 cores = 4 batches x 2 bucket-halves (c in [0,128) / [128,256)).
Each core computes a partial gather (zero where the query's code is in the
other half); host sums the pair and subtracts 1 (table stores j+1, empty=0).

v5 structure:
- inputs host-binarized to fp16 {0,1}; no device binarize, and the hash
  matmuls start once the (halved, 2-queue) kT DMAs land.
- k-side codes: 0/1 bits via DVE is_ge from psum (the zero-padded psum rows
  become free all-ones rows); bucket match is Relu(agree + 1 - popcount(c))
  with a per-partition bias (no ACT Sign on the k critical path).
- the scatter-index mask is fused to ONE DVE op per half:
  idx = (rank - 1) + T, where T = 2048*(sum_h bit_h*pm - popcount) <= 0 is
  built by one extra matmul per chunk (pm rows scaled by 2048 plus a
  -2048*popcount coefficient on the free ones-row). T = 0 exactly at
  matches and <= -2048 otherwise, so non-matched keys go negative (ignored
  by local_scatter; ranks <= 2048 keep everything inside int16).
- fp16 iota -> fp16 tables; the per-query gather accumulates
  q1h^T @ tab0 + q1h^T @ tab1 in psum (one accumulation group per psum
  bank), with tab0 matmuls issued before scatter h1 completes.
- psum copied out in two halves (ACT + DVE) feeding SP + ACT DMA queues.

Precision: the hash sign test needs ~f32-accurate projections. W is split
as fp16(W) + fp16(W - fp16(W)) and the two fp16 matmuls accumulate in f32
PSUM; representation error ~1e-6 vs hash sign margins ~1e-4 on this data.
"""

import numpy as np

B, L, D, NH = 4, 2048, 64, 8
KMAX = 64
TABLE_ELEMS = 1024  # local_scatter num_elems; must exceed max bucket count
MPAD = 48           # hash matmul lhsT free size: 8 real + 40 zero rows
HALF = L // 2
BIG = 2048.0        # T-mask scale; > max rank, and 8*BIG stays in int16
# const-pack columns: wpk [0:96], sgnc [96:224], biasc [224], tpk [225:353]
CW, CS, CB, CT = 0, 2 * MPAD, 2 * MPAD + 128, 2 * MPAD + 129
CPACK = CT + 128

_cache = {}


def _build_program():
    import concourse.bass as bass
    import concourse.mybir as mybir
    from concourse import bacc, tile
    from contextlib import ExitStack

    dt = mybir.dt
    Alu = mybir.AluOpType
    Act = mybir.ActivationFunctionType

    nc = bacc.Bacc("TRN2", target_bir_lowering=False, debug=False)

    # DRAM I/O (per-core shapes); qT/kT are host-binarized {0,1} fp16
    qT_d = nc.declare_dram_parameter("qT", [D, L], dt.float16, isOutput=False)
    kT_d = nc.declare_dram_parameter("kT", [D, L], dt.float16, isOutput=False)
    cpk_d = nc.declare_dram_parameter("cpk", [128, CPACK], dt.float16, isOutput=False)
    out_d = nc.declare_dram_parameter("out", [L, KMAX], dt.float16, isOutput=True)

    with ExitStack() as ctx:
        tc = ctx.enter_context(tile.TileContext(nc))
        sb = ctx.enter_context(tc.tile_pool(name="sb", bufs=1))
        ps = ctx.enter_context(tc.tile_pool(name="ps", bufs=2, space="PSUM"))
        aps = ctx.enter_context(tc.tile_pool(name="aps", bufs=3, space="PSUM"))

        # ---- loads ----
        kT_sb = sb.tile([D, L], dt.float16, tag="kT")
        qT_sb = sb.tile([D, L], dt.float16, tag="qT")
        cpk_sb = sb.tile([128, CPACK], dt.float16, tag="cpk")
        biasc_sb = sb.tile([128, 1], dt.float32, tag="biasc")

        nc.sync.dma_start(kT_sb[:, 0:HALF], kT_d[:, 0:HALF])
        nc.sync.dma_start(qT_sb[:, 0:HALF], qT_d[:, 0:HALF])

        nc.gpsimd.dma_start(cpk_sb[:], cpk_d[:])
        nc.gpsimd.dma_start(kT_sb[:, HALF:L], kT_d[:, HALF:L])
        nc.gpsimd.dma_start(qT_sb[:, HALF:L], qT_d[:, HALF:L])

        wpk_sb = cpk_sb[0:D, CW : CW + 2 * MPAD]
        sgnc_sb = cpk_sb[0:MPAD, CS : CS + 128]
        tpk_sb = cpk_sb[0:MPAD, CT : CT + 128]

        # biasc as f32 (activation bias operand); tiny convert off cpk
        nc.vector.tensor_copy(biasc_sb[:], cpk_sb[:, CB : CB + 1])

        # bias constant for the q-side one-hot Relu(agree - 7)
        bias7 = sb.tile([128, 1], dt.float32, tag="bias7")
        nc.gpsimd.memset(bias7[:], -7.0)

        # iota data for the scatter: each partition holds 1..L; fp16 so the
        # scattered tables feed the gather matmuls directly (ints <= 2048
        # are exact in fp16).
        iota_sb = sb.tile([128, L], dt.float16, tag="iota")
        nc.gpsimd.iota(
            iota_sb[:], pattern=[[1, L]], base=1, channel_multiplier=0,
            allow_small_or_imprecise_dtypes=True,
        )



        # ---- PE warm-up: keep the tensor engine busy from t~1us so the
        # p-state ramp completes before the real matmuls arrive ----
        warm_src = sb.tile([D, 512], dt.float16, tag="warm")
        nc.vector.memset(warm_src[:], 0.0)
        wp = ps.tile([MPAD, 512], dt.float32, tag="hp")
        for r in range(4):
            nc.tensor.matmul(
                wp[:], lhsT=warm_src[:, 0:MPAD], rhs=warm_src[:],
                start=True, stop=True,
            )
        warm_sink = sb.tile([1, 1], dt.float32, tag="warmsink")
        nc.vector.tensor_copy(warm_sink[:], wp[0:1, 0:1])

        # ---- hash matmuls (shared helper): two fp16 matmuls (W-hi + W-lo)
        # accumulate into f32 psum; chunk pairs share a [64, 512] psum tile
        # at partition bases {0, 32}; base-0 uses zero-padded M=48 weights so
        # rows 8-31 and 40-47 are defined for the batched bits/sign op.
        def hash_pair(src_sb, g):
            t = ps.tile([64, 512], dt.float32, tag="hp")
            for u in range(2):
                c = 2 * g + u
                m = MPAD if u == 0 else NH
                nc.tensor.matmul(
                    t[32 * u : 32 * u + m, :],
                    lhsT=wpk_sb[:, 0:m], rhs=src_sb[:, 512 * c : 512 * (c + 1)],
                    start=True, stop=False,
                )
                nc.tensor.matmul(
                    t[32 * u : 32 * u + m, :],
                    lhsT=wpk_sb[:, MPAD : MPAD + m],
                    rhs=src_sb[:, 512 * c : 512 * (c + 1)],
                    start=False, stop=True,
                )
            return t

        onehot = sb.tile([128, L], dt.float16, tag="onehot")
        q1h = sb.tile([128, L], dt.float16, tag="q1h")

        # k side: hash -> DVE 0/1 bits (is_ge: psum padding rows become
        # all-ones) -> agree -> Relu(agree + 1 - popcount)
        kbits = []
        kapt = []
        for g in range(2):
            t = hash_pair(kT_sb, g)
            bits = sb.tile([MPAD, 512], dt.float16, tag=f"bitsk{g}")
            nc.vector.tensor_single_scalar(bits[:], t[0:MPAD, :], 0.0, Alu.is_ge)
            kbits.append(bits)
        for g in range(2):
            apt = aps.tile([128, 1024], dt.float32, tag="agree")
            kapt.append(apt)
            for u in range(2):
                nc.tensor.matmul(
                    apt[:, 512 * u : 512 * (u + 1)],
                    lhsT=sgnc_sb[32 * u : 32 * u + 8, :],
                    rhs=kbits[g][32 * u : 32 * u + 8, :],
                    start=True, stop=True,
                )
            nc.scalar.activation(
                onehot[:, 1024 * g : 1024 * (g + 1)], apt[:],
                Act.Relu, bias=biasc_sb[:],
            )

        # ---- rank keys within bucket (inclusive prefix sum along j) on DVE
        # (the scan opcode only exists on DVE), halves chained via the last
        # column of half 0; m1 = onehot*rank then idx = m1 - 1 (-1 at
        # non-matches = ignored by local_scatter). Half 0's mask + scatter
        # run between the two scans so scatter h0 clears the Pool queue
        # while scan h1 is still going.
        rank = sb.tile([128, L], dt.float16, tag="rank")
        m1 = sb.tile([128, L], dt.float16, tag="m1")
        idx16 = sb.tile([128, L], dt.int16, tag="idx16")
        tabs = []
        for h in range(2):
            lo, hi = HALF * h, HALF * (h + 1)
            init = 0.0 if h == 0 else rank[:, HALF - 1 : HALF]
            nc.vector.tensor_tensor_scan(
                rank[:, lo:hi], onehot[:, lo:hi], onehot[:, lo:hi],
                init, Alu.add, Alu.bypass,
            )
            # half 0's mul runs on the otherwise-idle Pool engine so DVE can
            # go straight from scan h0 into scan h1
            if h == 0:
                nc.gpsimd.tensor_mul(m1[:, lo:hi], onehot[:, lo:hi], rank[:, lo:hi])
            else:
                nc.vector.tensor_mul(m1[:, lo:hi], onehot[:, lo:hi], rank[:, lo:hi])
            nc.vector.tensor_single_scalar(
                idx16[:, lo:hi], m1[:, lo:hi], 1.0, Alu.subtract
            )
            tab = sb.tile([128, TABLE_ELEMS], dt.float16, tag=f"table{h}")
            tabs.append(tab)
            nc.gpsimd.local_scatter(
                tab[:], iota_sb[:, lo:hi], idx16[:, lo:hi],
                channels=128, num_elems=TABLE_ELEMS, num_idxs=HALF,
            )

        # q side: hash -> ACT Sign (+-1) -> agree -> Relu(agree - 7); the q
        # pair-1 agree matmuls are held until the h0 mask frees its psum
        # slot (aps bufs=3).
        from concourse.tile_rust import add_dep_helper  # noqa: E402

        qsgn = []
        for g in range(2):
            t = hash_pair(qT_sb, g)
            s = sb.tile([MPAD, 512], dt.float16, tag=f"sgnq{g}")
            nc.scalar.activation(s[:], t[0:MPAD, :], Act.Sign)
            qsgn.append(s)
        for g in range(2):
            apt = aps.tile([128, 1024], dt.float32, tag="agree")
            for u in range(2):
                nc.tensor.matmul(
                    apt[:, 512 * u : 512 * (u + 1)],
                    lhsT=sgnc_sb[32 * u : 32 * u + 8, :],
                    rhs=qsgn[g][32 * u : 32 * u + 8, :],
                    start=True, stop=True,
                )
            nc.scalar.activation(
                q1h[:, 1024 * g : 1024 * (g + 1)], apt[:], Act.Relu, bias=bias7[:]
            )

        # ---- gather per query: out[i, s] = sum_c q1h[c, i] * tab[c, s],
        # accumulating tab0 + tab1 in psum. Queries are chunked mod-8 within
        # each 1024-half so each half only needs that half's q one-hot and
        # the output DMA rows stay per-partition contiguous. Each half has
        # its own 1-bank psum tile = one accumulation group.
        HO = 8 * KMAX
        opA = ps.tile([128, HO], dt.float32, tag="hp")
        opB = ps.tile([128, HO], dt.float32, tag="hp")
        opt = [opA, opB]
        qv = [
            q1h[:, HALF * h : HALF * (h + 1)].rearrange("c (i t) -> c t i", t=8)
            for h in range(2)
        ]
        for h in range(2):
            for ti, tab in ((0, tabs[0]), (1, tabs[1])):
                for u in range(8):
                    nc.tensor.matmul(
                        opt[h][:, KMAX * u : KMAX * (u + 1)],
                        lhsT=qv[h][:, u, :], rhs=tab[:, 0:KMAX],
                        start=(ti == 0 and u == 0), stop=(ti == 1 and u == 7),
                    )
        out_sbA = sb.tile([128, HO], dt.float16, tag="out_sbA")
        out_sbB = sb.tile([128, HO], dt.float16, tag="out_sbB")
        out_v = [
            out_d[HALF * h : HALF * (h + 1), :].rearrange("(p t) s -> p (t s)", t=8)
            for h in range(2)
        ]
        # half A (closes first): DVE copy + SP queue; half B: ACT copy + ACT
        # queue (both engines are free by the time the banks close)
        nc.vector.tensor_copy(out_sbA[:], opt[0][:])
        nc.sync.dma_start(out_v[0][:], out_sbA[:])
        nc.scalar.activation(out_sbB[:], opt[1][:], Act.Copy)
        nc.scalar.dma_start(out_v[1][:], out_sbB[:])

    nc.compile()
    return nc


def _get_nc():
    if "nc" not in _cache:
        _cache["nc"] = _build_program()
    return _cache["nc"]


def _make_in_maps(query, key, W):
    query = np.asarray(query, dtype=np.float32)
    key = np.asarray(key, dtype=np.float32)
    W = np.asarray(W, dtype=np.float32)
    qT = [
        np.ascontiguousarray((query[b].T > 0)).astype(np.float16) for b in range(B)
    ]
    kT = [
        np.ascontiguousarray((key[b].T > 0)).astype(np.float16) for b in range(B)
    ]

    wpk = np.zeros((D, 2 * MPAD), np.float16)
    wpk[:, :NH] = W.astype(np.float16)
    wpk[:, MPAD : MPAD + NH] = (W - wpk[:, :NH].astype(np.float32)).astype(np.float16)

    cpks = []
    for h in range(2):
        cg = 128 * h + np.arange(128)  # global bucket ids of this half
        bits = ((cg[None, :] >> np.arange(NH)[:, None]) & 1).astype(np.float32)
        pm = (2.0 * bits - 1.0).astype(np.float16)  # [8, 128]
        cnt = bits.sum(axis=0)  # popcount per bucket
        cpk = np.zeros((128, CPACK), np.float16)
        cpk[0:D, CW : CW + 2 * MPAD] = wpk
        cpk[0:NH, CS : CS + 128] = pm
        cpk[32 : 32 + NH, CS : CS + 128] = pm
        # match iff sum_h bit_h * pm_{h,c} == popcount(c):
        # relu(agree + 1 - popcount) is 1 at match, 0 otherwise
        cpk[:, CB] = (1.0 - cnt).astype(np.float16)
        # T-mask rows: BIG*pm on the bit rows, -BIG*popcount on the ones-row
        cpk[0:NH, CT : CT + 128] = (BIG * pm).astype(np.float16)
        cpk[NH, CT : CT + 128] = (-BIG * cnt).astype(np.float16)
        cpk[32 : 32 + NH, CT : CT + 128] = (BIG * pm).astype(np.float16)
        cpk[40, CT : CT + 128] = (-BIG * cnt).astype(np.float16)
        cpks.append(cpk)
    return [
        {"qT": qT[c // 2], "kT": kT[c // 2], "cpk": cpks[c % 2]}
        for c in range(2 * B)
    ]


def _combine(results):
    out = np.empty((B, L, KMAX), dtype=np.int64)
    for b in range(B):
        g = results[2 * b]["out"].astype(np.int64) + results[2 * b + 1]["out"].astype(
            np.int64
        )
        out[b] = g - 1
    return out


def _run_spmd(in_maps, **kwargs):
    from concourse.bass_utils import run_bass_kernel_spmd

    return run_bass_kernel_spmd(_get_nc(), in_maps, list(range(2 * B)), **kwargs)


def kernel(query, key, W, head_idx=0, **_unused):
    in_maps = _make_in_maps(query, key, W)
    res = _run_spmd(in_maps)
    return _combine(res.results)
